# revision 1
# baseline (speedup 1.0000x reference)
"""2-layer GCN encoder on 8 Trainium2 NeuronCores (Bass/Tile).

Math: with dis = deg^{-1/2} (self-loops included), the GCN layer
    out = relu(D^{-1/2} A D^{-1/2} (X W) + b)
separates as
    out[v] = relu(dis[v] * (sum_{e: dst=v} dis[src]*X[src]) @ W + b)
so the per-edge norm disappears and both weight matmuls commute out of the
edge aggregation.  Aggregation is done as binary-selection matmuls on the
TensorEngine over dma_gather'ed rows of the dis-prescaled feature table.

Sharding: nodes are bin-packed by in-degree into 8 cores x 112 groups x 128
slots, with slot residues balanced so that every (group, src%4) edge cell
fits in 4 tiles of 128 (the %4 chunking is needed because dma_gather indices
are int16).  Both layers share one table order (tix) and hence one schedule:
layer 1 gathers from the dis-prescaled x table, layer 2 from the AllGathered
dis-prescaled relu(h1) table laid out in the same order.  The AllGather is
split into uneven pieces pipelined behind layer-1 compute.

Device-side notes:
  - the one-hot selection matrix S3 is built on DVE with the iota operand in
    PSUM so the op uses only dedicated SBUF ports and never locks GpSimd
    (SWDGE descriptor generation) out of the shared port pair;
  - bias is pre-loaded into PSUM as outer(1/dis, b) via a K=1 matmul, and the
    dst-side dis scaling + relu ride the Scalar engine's activation
    (dis*relu(x) == relu(dis*x));
  - each chunk's gather is split into two half-calls issued wave-wise
    (h-major) so a descriptor-ring await on one SWDGE queue never blocks the
    other queues' in-order decode on the Pool engine.
"""

import numpy as np
import ml_dtypes

import concourse.bacc as bacc
import concourse.tile as tile
import concourse.mybir as mybir
import concourse.bass as bass
from concourse.bass_utils import run_bass_kernel_spmd

# problem shapes (hardcoded per contract)
N = 100000
E = 1600000
IN_DIM, HID, OUT_DIM = 128, 128, 64

# schedule constants
P = 128           # partitions / tile edge count
NC_ = 8           # cores
G = 112           # groups per core
W = 7             # groups per batch
NB = 16           # batches per layer (W*NB == G)
TPC = 4           # tiles per (group, chunk)
NSEC = 4          # chunks (src table row mod 4)
SEC_T = W * TPC   # tiles per chunk section      = 28
BT = NSEC * SEC_T # tiles per batch              = 112
NODES_PC = G * P  # padded nodes per core        = 14336
TROWS = NC_ * NODES_PC  # shared table rows      = 114688
# AllGather pieces: uneven batch boundaries so the tail piece is small and
# the next-to-last piece fires early enough to drain before layer-1 ends
PIECE_B = [5, 10, 13, 16]          # fire after batches 4, 9, 12, 15
PIECE_G = [0, 35, 70, 91, 112]     # group boundaries (batches * W)
HCALLS = 2              # gather half-calls per chunk (descriptor-ring pipelining)
HT = SEC_T // HCALLS    # tiles per half-call = 14
IDXH = HT * P // 16     # wrapped idx cols per half-call = 112
IDXB = NSEC * HCALLS * IDXH  # idx cols per batch = 896
CELL_CAP = TPC * P      # max edges per (group, chunk) cell = 512

BF16 = ml_dtypes.bfloat16

_compiled = None  # cache across calls


# ----------------------------------------------------------------- host side

def _pack_nodes(deg):
    """Bin-pack nodes into 8*G bins (<=128 nodes each), balancing in-degree.

    Returns bin_of [N] (bin id), counts [NBINS].
    """
    import heapq
    NBINS = NC_ * G
    order = np.argsort(-deg, kind="stable")
    counts = np.zeros(NBINS, np.int64)
    loads = np.zeros(NBINS, np.float64)
    bin_of = np.empty(N, np.int64)
    h = [(0.0, b) for b in range(NBINS)]
    heapq.heapify(h)
    for n in order:
        while True:
            load, b = heapq.heappop(h)
            if counts[b] < P:
                break
        bin_of[n] = b
        counts[b] += 1
        loads[b] = load + deg[n]
        if counts[b] < P:
            heapq.heappush(h, (loads[b], b))
    # bins -> (core, gabs): snake-assign by load so core totals balance
    bins_sorted = np.argsort(-loads, kind="stable")
    core_of_bin = np.empty(NBINS, np.int64)
    gabs_of_bin = np.empty(NBINS, np.int64)
    next_g = np.zeros(NC_, np.int64)
    for r, b in enumerate(bins_sorted):
        rnd, pos = divmod(r, NC_)
        core = pos if rnd % 2 == 0 else NC_ - 1 - pos
        core_of_bin[b] = core
        gabs_of_bin[b] = next_g[core]
        next_g[core] += 1
    return bin_of, core_of_bin, gabs_of_bin


def _balance_residues(src_all, dstbin, bin_of, rng):
    """Assign each node a slot residue (slot % 4) so that every
    (dst bin, src residue) cell has <= CELL_CAP edges, respecting the
    <=32 nodes-per-residue-per-bin capacity.

    Returns res [N] in 0..3.
    """
    NBINS = NC_ * G
    # initial: random-ish balanced within each bin
    res = rng.permutation(N) % NSEC
    # enforce per-bin residue caps (<=32) by re-dealing within each bin
    order = np.argsort(bin_of, kind="stable")
    res_sorted = np.empty(N, np.int64)
    start = 0
    binned_counts = np.bincount(bin_of, minlength=NBINS)
    for b in range(NBINS):
        cnt = binned_counts[b]
        res_sorted[start:start + cnt] = np.arange(cnt) % NSEC
        start += cnt
    res[order] = res_sorted

    # per-src out-bin multiplicities (CSR over unique (src, dstbin))
    key = src_all * np.int64(NBINS) + dstbin
    ukey, mult = np.unique(key, return_counts=True)
    usrc = ukey // NBINS
    ubin = ukey % NBINS
    ptr = np.searchsorted(usrc, np.arange(N + 1))

    loads = np.bincount(dstbin * NSEC + res[src_all],
                        minlength=NBINS * NSEC).astype(np.int64)
    rescnt = np.zeros((NBINS, NSEC), np.int64)
    np.add.at(rescnt, (bin_of, res), 1)

    # edges grouped by (dstbin, residue-of-src) for mover lookup
    ecell = dstbin * NSEC + res[src_all]
    eorder = np.argsort(ecell, kind="stable")
    estart = np.searchsorted(ecell[eorder], np.arange(NBINS * NSEC + 1))

    for _pass in range(6):
        bad = np.flatnonzero(loads > CELL_CAP)
        if len(bad) == 0:
            break
        for cell in bad:
            excess = loads[cell] - CELL_CAP
            if excess <= 0:
                continue
            r = cell % NSEC
            movers = np.unique(src_all[eorder[estart[cell]:estart[cell + 1]]])
            for u in movers:
                if excess <= 0:
                    break
                if res[u] != r:
                    continue
                ub = ubin[ptr[u]:ptr[u + 1]]
                um = mult[ptr[u]:ptr[u + 1]]
                mybin = bin_of[u]
                best_r2, best_pen = -1, None
                for r2 in range(NSEC):
                    if r2 == r or rescnt[mybin, r2] >= P // NSEC:
                        continue
                    newloads = loads[ub * NSEC + r2] + um
                    pen = np.maximum(newloads - CELL_CAP, 0).sum()
                    if pen == 0 and (best_pen is None or best_pen > 0):
                        best_r2, best_pen = r2, 0
                        break
                    if best_pen is None or pen < best_pen:
                        best_r2, best_pen = r2, pen
                if best_r2 < 0 or (best_pen is not None and best_pen > 0):
                    continue
                loads[ub * NSEC + r] -= um
                loads[ub * NSEC + best_r2] += um
                rescnt[mybin, r] -= 1
                rescnt[mybin, best_r2] += 1
                res[u] = best_r2
                moved = um[ub == (cell // NSEC)].sum()
                excess -= moved
        # refresh mover lookup after each pass
        ecell = dstbin * NSEC + res[src_all]
        eorder = np.argsort(ecell, kind="stable")
        estart = np.searchsorted(ecell[eorder], np.arange(NBINS * NSEC + 1))

    return res, loads


def preprocess(x, edge_index):
    src = np.asarray(edge_index[0], dtype=np.int64)
    dst = np.asarray(edge_index[1], dtype=np.int64)
    loops = np.arange(N, dtype=np.int64)
    src_all = np.concatenate([src, loops])
    dst_all = np.concatenate([dst, loops])
    deg = np.bincount(dst_all, minlength=N).astype(np.float64)
    dis = (1.0 / np.sqrt(deg)).astype(np.float32)

    bin_of, core_of_bin, gabs_of_bin = _pack_nodes(deg)
    node_core = core_of_bin[bin_of]
    node_gabs = gabs_of_bin[bin_of]

    rng = np.random.default_rng(12345)
    dstbin = bin_of[dst_all]
    res, loads = _balance_residues(src_all, dstbin, bin_of, rng)
    assert loads.max() <= CELL_CAP, f"cell overflow: {loads.max()}"

    # assign slots within each bin: residue r nodes take slots r, r+4, ...
    binres = bin_of * NSEC + res
    order = np.argsort(binres, kind="stable")
    rank_in_binres = np.empty(N, np.int64)
    sorted_br = binres[order]
    starts = np.searchsorted(sorted_br, np.arange(NC_ * G * NSEC + 1))
    rnk = np.arange(N) - np.repeat(starts[:-1], np.diff(starts))
    rank_in_binres[order] = rnk
    node_slot = res + NSEC * rank_in_binres
    assert node_slot.max() < P

    # shared table order: core-major, matching the whole-tensor AllGather
    gid = node_core * NODES_PC + node_gabs * P + node_slot
    tix = gid

    # layer-1 table in tix order
    xs = np.zeros((TROWS, IN_DIM), BF16)
    xs[tix] = (np.asarray(x, np.float32) * dis[:, None]).astype(BF16)

    # shared edge schedule (same for both layers)
    ecore = node_core[dst_all]
    egabs = node_gabs[dst_all]
    eslot = node_slot[dst_all]
    src_tix = tix[src_all]
    chunk = src_tix % NSEC
    eidx = src_tix // NSEC
    cell = (ecore * G + egabs) * NSEC + chunk
    order = np.lexsort((eidx, cell))
    cell_s = cell[order]
    counts = np.bincount(cell, minlength=NC_ * G * NSEC)
    assert counts.max() <= CELL_CAP
    starts = np.concatenate([[0], np.cumsum(counts)[:-1]])
    rank = np.arange(len(cell_s)) - np.repeat(starts, counts)
    ch = cell_s % NSEC
    gg = (cell_s // NSEC) % G
    cr = cell_s // (NSEC * G)
    batch = gg // W
    gslot = gg % W
    tile_k = rank // P
    pos = rank % P
    T = batch * BT + ch * SEC_T + gslot * TPC + tile_k
    goff = cr * (NB * BT * P) + T * P + pos
    flat_idx = np.zeros(NC_ * NB * BT * P, np.int16)
    flat_dl = np.full(NC_ * NB * BT * P, P, np.int16)
    flat_idx[goff] = eidx[order].astype(np.int16)
    flat_dl[goff] = eslot[order].astype(np.int16)

    # wrapped idx layout: wrapped[p, s] = flat[s*16 + p%16], replicated x8.
    # Per batch: NSEC*HCALLS half-call slices of HT*P idxs each.
    fi = flat_idx.reshape(NC_, NB * NSEC * HCALLS, IDXH, 16)
    A = fi.transpose(0, 3, 1, 2).reshape(NC_, 16, NB * IDXB)
    idx_dram = np.tile(A, (1, 8, 1))  # [8, 128, NB*IDXB]
    dl_dram = (
        flat_dl.reshape(NC_, NB * BT, P).transpose(0, 2, 1).astype(BF16)
    )  # [8, 128, NB*BT]

    dis_sb = np.zeros((NC_, P, G), np.float32)
    dis_sb[node_core, node_slot, node_gabs] = dis
    binv = np.zeros((NC_, G * P), np.float32)
    binv[node_core, node_gabs * P + node_slot] = 1.0 / dis

    return dict(
        xs=xs, idx=np.ascontiguousarray(idx_dram),
        dl=np.ascontiguousarray(dl_dram), dis_sb=dis_sb, binv=binv, gid=gid
    )


# --------------------------------------------------------------- device side

def build_program():
    f32 = mybir.dt.float32
    bf16 = mybir.dt.bfloat16
    i16 = mybir.dt.int16
    AO = mybir.AluOpType

    nc = bacc.Bacc(
        "TRN2", target_bir_lowering=False, debug=False, num_devices=NC_,
        num_swdge_queues=4, dynamic_dma_scratch_size=16384,
    )
    xs_d = nc.dram_tensor("xs", [TROWS, IN_DIM], bf16, kind="ExternalInput")
    idx_d = nc.dram_tensor("idx", [P, NB * IDXB], i16, kind="ExternalInput")
    dl_d = nc.dram_tensor("dl", [P, NB * BT], bf16, kind="ExternalInput")
    dis_d = nc.dram_tensor("dis", [P, G], f32, kind="ExternalInput")
    dis2_d = nc.dram_tensor("dis2", [P, G], f32, kind="ExternalInput")
    binv_d = nc.dram_tensor("binv", [1, G * P], bf16, kind="ExternalInput")
    w1_d = nc.dram_tensor("w1", [IN_DIM, HID], bf16, kind="ExternalInput")
    w2_d = nc.dram_tensor("w2", [HID, OUT_DIM], bf16, kind="ExternalInput")
    b1_d = nc.dram_tensor("b1w", [1, HID], bf16, kind="ExternalInput")
    b2_d = nc.dram_tensor("b2w", [1, OUT_DIM], bf16, kind="ExternalInput")
    iota_d = nc.dram_tensor("iota", [P, P], f32, kind="ExternalInput")
    out_d = nc.dram_tensor("out", [NODES_PC, OUT_DIM], f32, kind="ExternalOutput")

    with tile.TileContext(nc) as tc:
        with tc.tile_pool(name="const", bufs=1) as cpool, \
             tc.tile_pool(name="io", bufs=3) as iopool, \
             tc.tile_pool(name="msgp", bufs=3) as mpool, \
             tc.tile_pool(name="sp", bufs=3) as spool, \
             tc.tile_pool(name="epi", bufs=3) as epool, \
             tc.tile_pool(name="psag", bufs=3, space="PSUM") as psag, \
             tc.tile_pool(name="psep", bufs=2, space="PSUM") as psep, \
             tc.tile_pool(name="psio", bufs=1, space="PSUM") as psio, \
             tc.tile_pool(name="dram", bufs=1, space="DRAM") as dpool:

            w1s = cpool.tile([IN_DIM, HID], bf16)
            nc.sync.dma_start(out=w1s[:], in_=w1_d[:])
            w2s = cpool.tile([HID, OUT_DIM], bf16)
            nc.sync.dma_start(out=w2s[:], in_=w2_d[:])
            b1s = cpool.tile([1, HID], bf16)
            nc.sync.dma_start(out=b1s[:], in_=b1_d[:])
            b2s = cpool.tile([1, OUT_DIM], bf16)
            nc.sync.dma_start(out=b2s[:], in_=b2_d[:])
            binv_s = cpool.tile([1, G * P], bf16)
            nc.sync.dma_start(out=binv_s[:], in_=binv_d[:])
            dis_s = cpool.tile([P, G], f32)
            nc.sync.dma_start(out=dis_s[:], in_=dis_d[:])
            dis2_s = cpool.tile([P, G], f32)
            nc.sync.dma_start(out=dis2_s[:], in_=dis2_d[:])
            iota_s = cpool.tile([P, P], f32)
            nc.sync.dma_start(out=iota_s[:], in_=iota_d[:])
            iota_ps = psio.tile([P, P], f32)
            nc.scalar.copy(out=iota_ps[:], in_=iota_s[:])

            gshard = dpool.tile([NODES_PC, HID], bf16)
            gf_p = [
                dpool.tile(
                    [NC_ * (PIECE_G[k + 1] - PIECE_G[k]) * P, HID], bf16,
                    addr_space="Shared", name=f"gfp{k}")
                for k in range(len(PIECE_B))
            ]
            gfull = dpool.tile([TROWS, HID], bf16)

            xs_v = xs_d[:].rearrange("(n f) d -> n f d", f=NSEC)
            gf_v = gfull.rearrange("(n f) d -> n f d", f=NSEC)
            gfull_cv = gfull.rearrange("(c q) d -> c q d", c=NC_)

            def layer(tbl_view, wsb, bsb, dout, sink, post_batch=None):
                for b in range(NB):
                    idx_t = iopool.tile([P, IDXB], i16, tag="idx")
                    nc.sync.dma_start(
                        out=idx_t[:], in_=idx_d[:, b * IDXB:(b + 1) * IDXB]
                    )
                    dl_t = iopool.tile([P, BT], bf16, tag="dl")
                    nc.sync.dma_start(out=dl_t[:], in_=dl_d[:, b * BT:(b + 1) * BT])
                    msg = mpool.tile([P, BT, P], bf16, tag="msg")
                    for h in range(HCALLS):
                        for c in range(NSEC):
                            t0 = c * SEC_T + h * HT
                            s0 = (c * HCALLS + h) * IDXH
                            nc.gpsimd.dma_gather(
                                out_ap=msg[:, t0:t0 + HT, :],
                                in_ap=tbl_view[:, c, :],
                                idxs_ap=idx_t[:, s0:s0 + IDXH],
                                num_idxs=HT * P,
                                num_idxs_reg=HT * P,
                                elem_size=IN_DIM,
                                elem_step=IN_DIM * NSEC,
                                single_packet=False,
                                queue_num=c,
                            )
                    S3 = spool.tile([P, BT, P], bf16, tag="S3")
                    nc.vector.tensor_tensor(
                        out=S3[:],
                        in0=dl_t[:].unsqueeze(2).to_broadcast([P, BT, P]),
                        in1=iota_ps[:].unsqueeze(1).to_broadcast([P, BT, P]),
                        op=AO.is_equal,
                    )
                    for g in range(W):
                        gabs = b * W + g
                        ps = psag.tile([P, P], mybir.dt.float32, tag="agg")
                        for c in range(NSEC):
                            for k in range(TPC):
                                t = c * SEC_T + g * TPC + k
                                nc.tensor.matmul(
                                    out=ps[:],
                                    lhsT=msg[:, t, :],
                                    rhs=S3[:, t, :],
                                    start=(c == 0 and k == 0),
                                    stop=(c == NSEC - 1 and k == TPC - 1),
                                )
                        aggT = epool.tile([P, P], bf16, tag="aggT")
                        nc.scalar.copy(out=aggT[:], in_=ps[:])
                        po = psep.tile([P, dout], mybir.dt.float32, tag="po")
                        # bias pre-load: po = (b / dis)[dst, f] via outer product
                        nc.tensor.matmul(
                            out=po[:],
                            lhsT=binv_s[:, gabs * P:(gabs + 1) * P],
                            rhs=bsb[:],
                            start=True, stop=False,
                        )
                        nc.tensor.matmul(
                            out=po[:], lhsT=aggT[:], rhs=wsb[:], start=False, stop=True
                        )
                        sink(gabs, po)
                    if post_batch is not None:
                        post_batch(b)

            def sink1(gabs, po):
                # dis*relu(dis*agg + b1) == relu(dis2*(agg + b1/dis))
                gt = epool.tile([P, HID], mybir.dt.bfloat16, tag="gt")
                nc.scalar.activation(
                    out=gt[:], in_=po[:],
                    func=mybir.ActivationFunctionType.Relu,
                    scale=dis2_s[:, gabs:gabs + 1],
                )
                nc.sync.dma_start(
                    out=gshard[gabs * P:(gabs + 1) * P, :], in_=gt[:]
                )

            def sink2(gabs, po):
                # dis*agg + b2 == Copy(dis*(agg + b2/dis))
                o = epool.tile([P, OUT_DIM], mybir.dt.float32, tag="o")
                nc.scalar.activation(
                    out=o[:], in_=po[:],
                    func=mybir.ActivationFunctionType.Copy,
                    scale=dis_s[:, gabs:gabs + 1],
                )
                nc.sync.dma_start(
                    out=out_d[gabs * P:(gabs + 1) * P, :], in_=o[:]
                )

            def ag_piece(b):
                # fire AllGather piece k once its groups are sunk, then
                # scatter the piece into the core-major local table copy
                if (b + 1) not in PIECE_B:
                    return
                k = PIECE_B.index(b + 1)
                r0, r1 = PIECE_G[k] * P, PIECE_G[k + 1] * P
                nc.gpsimd.collective_compute(
                    "AllGather",
                    mybir.AluOpType.bypass,
                    replica_groups=[list(range(NC_))],
                    ins=[gshard[r0:r1, :].opt()],
                    outs=[gf_p[k].opt()],
                )
                src_v = gf_p[k].rearrange("(c q) d -> c q d", c=NC_)
                nc.sync.dma_start(
                    out=gfull_cv[:, r0:r1, :],
                    in_=src_v[:],
                )

            layer(xs_v, w1s, b1s, HID, sink1, post_batch=ag_piece)
            layer(gf_v, w2s, b2s, OUT_DIM, sink2)

    nc.compile()
    return nc


# ------------------------------------------------------------------- runner

def run(inputs, trace=False):
    global _compiled
    x = np.asarray(inputs["x"], np.float32)
    edge_index = np.asarray(inputs["edge_index"])
    W1 = np.asarray(inputs["W1"], np.float32)
    b1 = np.asarray(inputs["b1"], np.float32)
    W2 = np.asarray(inputs["W2"], np.float32)
    b2 = np.asarray(inputs["b2"], np.float32)

    pp = preprocess(x, edge_index)

    if _compiled is None:
        _compiled = build_program()
    nc = _compiled

    iota = np.ascontiguousarray(
        np.broadcast_to(np.arange(P, dtype=np.float32), (P, P))
    )
    w1b = W1.astype(BF16)
    w2b = W2.astype(BF16)
    b1w = b1.reshape(1, HID).astype(BF16)
    b2w = b2.reshape(1, OUT_DIM).astype(BF16)

    in_maps = []
    for c in range(NC_):
        in_maps.append({
            "xs": pp["xs"],
            "idx": pp["idx"][c],
            "dl": pp["dl"][c],
            "dis": pp["dis_sb"][c],
            "dis2": pp["dis_sb"][c] ** 2,
            "binv": pp["binv"][c].reshape(1, G * P).astype(BF16),
            "w1": w1b,
            "w2": w2b,
            "b1w": b1w,
            "b2w": b2w,
            "iota": iota,
        })

    res = run_bass_kernel_spmd(
        nc, in_maps, core_ids=list(range(NC_)), trace=trace
    )
    allf = np.concatenate([res.results[c]["out"] for c in range(NC_)], axis=0)
    out = allf[pp["gid"]].astype(np.float32)
    return out, res


def kernel(**inputs):
    out, _ = run(inputs, trace=False)
    return out



# revision 12
# speedup vs baseline: 1.0515x; 1.0515x over previous
"""2-layer GCN encoder on 8 Trainium2 NeuronCores (Bass/Tile).

Math: with dis = deg^{-1/2} (self-loops included), the GCN layer
    out = relu(D^{-1/2} A D^{-1/2} (X W) + b)
separates as
    out[v] = relu(dis[v] * (sum_{e: dst=v} dis[src]*X[src]) @ W + b)
so the per-edge norm disappears and both weight matmuls commute out of the
edge aggregation.  Aggregation is done as binary-selection matmuls on the
TensorEngine over dma_gather'ed rows of the dis-prescaled feature table.

Sharding: nodes are bin-packed by in-degree into 8 cores x 104 groups x 128
slots.  Groups are partitioned into Q=4 quarters of 26 groups per core; a
node's gather-chunk is the quarter of its group, and the (dst group, src
quarter) cell loads are balanced to <= 512 edges by bin- and node-level
swaps so every cell fits TPC=4 tiles of 128.  The table is quarter-major:
each quarter is one contiguous sub-table of 26624 rows (int16-indexable),
and the layer-2 table is four Shared DRAM tensors, each written in place by
exactly one pipelined AllGather piece -- no post-collective scatter.

Self-loops are excluded from the edge schedule: each group's own rows are
bulk-loaded transposed (one xbar-transpose DMA per batch) and accumulated
into the aggregation PSUM via an identity-matrix matmul, saving ~6% of the
per-element gather descriptors.

Device-side notes:
  - the one-hot selection matrix S3 is built on DVE with the iota operand in
    PSUM so the op uses only dedicated SBUF ports and never locks GpSimd
    (SWDGE descriptor generation) out of the shared port pair;
  - bias is pre-loaded into PSUM as outer(1/dis, b) via a K=1 matmul, and the
    dst-side dis scaling + relu ride the Scalar engine's activation
    (dis*relu(x) == relu(dis*x));
  - each chunk's gather is split into two half-calls issued wave-wise
    (h-major) so a descriptor-ring await on one SWDGE queue never blocks the
    other queues' in-order decode on the Pool engine.
"""

import numpy as np
import ml_dtypes

import concourse.bacc as bacc
import concourse.tile as tile
import concourse.mybir as mybir
import concourse.bass as bass
from concourse.bass_utils import run_bass_kernel_spmd

# problem shapes (hardcoded per contract)
N = 100000
E = 1600000
IN_DIM, HID, OUT_DIM = 128, 128, 64

# schedule constants
P = 128           # partitions / tile edge count
NC_ = 8           # cores
G = 104           # groups per core
W = 8             # groups per batch
NB = 13           # batches per layer (W*NB == G)
Q = 4             # quarters (gather chunks == AllGather pieces)
GQ = G // Q       # groups per quarter per core  = 26
TPC = 4           # tiles per (group, chunk) cell
CELL_CAP = TPC * P  # max edges per (dst group, src quarter) cell = 512
SEC_T = W * TPC   # tiles per chunk section per batch = 32
BT = Q * SEC_T    # tiles per batch                   = 128
NODES_PC = G * P  # padded nodes per core             = 13312
QROWS = NC_ * GQ * P    # rows per quarter sub-table  = 26624
TROWS = NC_ * NODES_PC  # shared table rows           = 106496
HCALLS = 2              # gather half-calls per chunk
HT = SEC_T // HCALLS    # tiles per half-call = 16
IDXH = HT * P // 16     # wrapped idx cols per half-call = 128
IDXB = Q * HCALLS * IDXH  # idx cols per batch = 1024
# AllGather piece k (== quarter k) fires once batches covering its 26 groups
# are sunk: after batch indices 3, 6, 9, 12
PIECE_B = [4, 7, 10, 13]

BF16 = ml_dtypes.bfloat16

_compiled = None  # cache across calls


# ----------------------------------------------------------------- host side

def _pack_and_balance(src, dst, deg_in):
    """Assign nodes to 8*G bins (<=128 each) and bins to (core, quarter) so
    that every (dst bin, src quarter) cell has <= CELL_CAP edges.

    Returns bin_of [N], core_of_bin, quarter_of_bin [NBINS].
    """
    NBINS = NC_ * G
    rng = np.random.default_rng(0)

    # snake-deal nodes by in-degree: balances per-bin degree loads
    order = np.argsort(-deg_in, kind="stable")
    seq = np.arange(N) % (2 * NBINS)
    binpos = np.where(seq < NBINS, seq, 2 * NBINS - 1 - seq)
    bin_of = np.empty(N, np.int64)
    bin_of[order] = binpos
    counts = np.bincount(bin_of, minlength=NBINS)
    assert counts.max() <= P

    core_of_bin = np.arange(NBINS) % NC_
    within = np.arange(NBINS) // NC_
    qq = within % (2 * Q)
    qob = np.where(qq < Q, qq, 2 * Q - 1 - qq)

    B = np.zeros((NBINS, NBINS), np.int32)
    np.add.at(B, (bin_of[src], bin_of[dst]), 1)

    def cellmat():
        c = np.zeros((Q, NBINS), np.int64)
        for q in range(Q):
            c[q] = B[qob == q].sum(axis=0)
        return c

    c = cellmat()

    # --- bin phase: swap quarters of two same-core bins ---
    for _ in range(5000):
        exc = np.maximum(c - CELL_CAP, 0)
        if exc.sum() == 0:
            break
        q, dd = np.unravel_index(np.argmax(exc), exc.shape)
        cand = np.flatnonzero(qob == q)
        cand = cand[np.argsort(-B[cand, dd])][:3]
        best = (0, -1, -1)
        for b in cand:
            mates = np.flatnonzero((core_of_bin == core_of_bin[b]) & (qob != q))
            rowb = B[b]
            for b2 in mates:
                q2 = qob[b2]
                diff = rowb - B[b2]
                delta = (
                    np.maximum(c[q] - diff - CELL_CAP, 0).sum()
                    + np.maximum(c[q2] + diff - CELL_CAP, 0).sum()
                    - exc[q].sum() - exc[q2].sum()
                )
                if delta < best[0]:
                    best = (delta, b, b2)
        if best[1] < 0:
            break
        _, b, b2 = best
        q2 = qob[b2]
        diff = B[b] - B[b2]
        c[q] -= diff
        c[q2] += diff
        qob[b], qob[b2] = q2, q

    # --- node phase: swap two nodes between bins of different quarters ---
    os_ = np.argsort(src, kind="stable")
    osrc_sorted = src[os_]
    odst_node = dst[os_]
    optr = np.searchsorted(osrc_sorted, np.arange(N + 1))
    is_ = np.argsort(dst, kind="stable")
    idst_sorted = dst[is_]
    isrc_node = src[is_]
    iptr = np.searchsorted(idst_sorted, np.arange(N + 1))

    nodes_by_bin = [np.flatnonzero(bin_of == b) for b in range(NBINS)]

    def try_swap(u, v2, c, strict):
        bu, b2 = bin_of[u], bin_of[v2]
        q, q2 = qob[bu], qob[b2]
        du = bin_of[odst_node[optr[u]:optr[u + 1]]]
        dv = bin_of[odst_node[optr[v2]:optr[v2 + 1]]]
        iu = bin_of[isrc_node[iptr[u]:iptr[u + 1]]]
        iv = bin_of[isrc_node[iptr[v2]:iptr[v2 + 1]]]
        cn = c.copy()
        np.add.at(cn[q], du, -1)
        np.add.at(cn[q2], du, 1)
        np.add.at(cn[q2], dv, -1)
        np.add.at(cn[q], dv, 1)
        np.add.at(cn, (qob[iu], np.full(len(iu), bu)), -1)
        np.add.at(cn, (qob[iu], np.full(len(iu), b2)), 1)
        np.add.at(cn, (qob[iv], np.full(len(iv), b2)), -1)
        np.add.at(cn, (qob[iv], np.full(len(iv), bu)), 1)
        d = np.maximum(cn - CELL_CAP, 0).sum() - np.maximum(c - CELL_CAP, 0).sum()
        if d < 0 or (not strict and d == 0):
            return cn
        return None

    moves = 0
    for it in range(60000):
        exc = np.maximum(c - CELL_CAP, 0)
        tot = exc.sum()
        if tot == 0:
            break
        q, dd = np.unravel_index(np.argmax(exc), exc.shape)
        # src nodes feeding cell (dd, q), weighted by multiplicity
        cnt = {}
        for v in nodes_by_bin[dd]:
            for e in range(iptr[v], iptr[v + 1]):
                u = isrc_node[e]
                if qob[bin_of[u]] == q:
                    cnt[u] = cnt.get(u, 0) + 1
        us = sorted(cnt, key=cnt.get, reverse=True)[:6]
        done = False
        for strict in (True, False):
            for u in us:
                q2s = np.argsort(c[:, dd])
                for q2 in q2s:
                    if q2 == q:
                        continue
                    tb = np.flatnonzero(qob == q2)
                    for b2 in rng.choice(tb, size=min(6, len(tb)), replace=False):
                        vc = nodes_by_bin[b2]
                        v2 = vc[np.argmin(np.abs(deg_in[vc] - deg_in[u]))]
                        cn = try_swap(u, v2, c, strict)
                        if cn is not None:
                            bu, b2_ = bin_of[u], bin_of[v2]
                            c = cn
                            bin_of[u], bin_of[v2] = b2_, bu
                            nodes_by_bin[bu] = nodes_by_bin[bu][nodes_by_bin[bu] != u]
                            nodes_by_bin[bu] = np.append(nodes_by_bin[bu], v2)
                            nodes_by_bin[b2_] = nodes_by_bin[b2_][nodes_by_bin[b2_] != v2]
                            nodes_by_bin[b2_] = np.append(nodes_by_bin[b2_], u)
                            moves += 1
                            done = True
                            break
                    if done:
                        break
                if done:
                    break
            if done:
                break
        if not done:
            break
        if moves % 256 == 0:
            # guard against incremental drift (adjacent-node swaps)
            Bn = np.zeros((NBINS, NBINS), np.int32)
            np.add.at(Bn, (bin_of[src], bin_of[dst]), 1)
            B = Bn
            c = cellmat()

    # authoritative final check
    B = np.zeros((NBINS, NBINS), np.int32)
    np.add.at(B, (bin_of[src], bin_of[dst]), 1)
    c = cellmat()
    assert np.maximum(c - CELL_CAP, 0).sum() == 0, (
        f"cell balance failed: max={c.max()}"
    )
    return bin_of, core_of_bin, qob


def preprocess(x, edge_index):
    src = np.asarray(edge_index[0], dtype=np.int64)
    dst = np.asarray(edge_index[1], dtype=np.int64)
    deg = np.bincount(dst, minlength=N).astype(np.float64) + 1.0  # + self-loop
    dis = (1.0 / np.sqrt(deg)).astype(np.float32)

    deg_in = deg.astype(np.int64)
    bin_of, core_of_bin, qob = _pack_and_balance(src, dst, deg_in)
    NBINS = NC_ * G

    # order bins within each (core, quarter) -> gabs
    gabs_of_bin = np.empty(NBINS, np.int64)
    for core in range(NC_):
        for q in range(Q):
            sel = np.flatnonzero((core_of_bin == core) & (qob == q))
            assert len(sel) == GQ
            gabs_of_bin[sel] = q * GQ + np.arange(GQ)

    node_core = core_of_bin[bin_of]
    node_gabs = gabs_of_bin[bin_of]
    # slot within bin: arbitrary stable order
    bkey = bin_of
    order = np.argsort(bkey, kind="stable")
    sorted_b = bkey[order]
    starts = np.searchsorted(sorted_b, np.arange(NBINS + 1))
    rnk = np.arange(N) - np.repeat(starts[:-1], np.diff(starts))
    node_slot = np.empty(N, np.int64)
    node_slot[order] = rnk
    assert node_slot.max() < P

    # output gather order: core-major (matches per-core out tensors)
    gid = node_core * NODES_PC + node_gabs * P + node_slot
    # shared table order: quarter-major so each AllGather piece (quarter) is
    # one contiguous sub-table written in place by its collective
    nq = node_gabs // GQ
    tix = (
        nq * QROWS
        + node_core * (GQ * P)
        + (node_gabs - nq * GQ) * P
        + node_slot
    )

    # layer-1 table in tix order + per-core own-block table in gabs order
    xpre = (np.asarray(x, np.float32) * dis[:, None]).astype(BF16)
    xs = np.zeros((TROWS, IN_DIM), BF16)
    xs[tix] = xpre
    xs_own = np.zeros((NC_, NODES_PC, IN_DIM), BF16)
    xs_own[node_core, node_gabs * P + node_slot] = xpre

    # shared edge schedule (same for both layers); self-loops excluded
    ecore = node_core[dst]
    egabs = node_gabs[dst]
    eslot = node_slot[dst]
    src_tix = tix[src]
    chunk = src_tix // QROWS
    eidx = src_tix % QROWS
    cell = (ecore * G + egabs) * Q + chunk
    order = np.lexsort((eidx, cell))
    cell_s = cell[order]
    counts = np.bincount(cell, minlength=NBINS * Q)
    assert counts.max() <= CELL_CAP
    starts = np.concatenate([[0], np.cumsum(counts)[:-1]])
    rank = np.arange(len(cell_s)) - np.repeat(starts, counts)
    ch = cell_s % Q
    gg = (cell_s // Q) % G
    cr = cell_s // (Q * G)
    batch = gg // W
    gslot = gg % W
    tile_k = rank // P
    pos = rank % P
    T = batch * BT + ch * SEC_T + gslot * TPC + tile_k
    goff = cr * (NB * BT * P) + T * P + pos
    flat_idx = np.zeros(NC_ * NB * BT * P, np.int16)
    flat_dl = np.full(NC_ * NB * BT * P, P, np.int16)
    flat_idx[goff] = eidx[order].astype(np.int16)
    flat_dl[goff] = eslot[order].astype(np.int16)

    # wrapped idx layout: wrapped[p, s] = flat[s*16 + p%16], replicated x8.
    fi = flat_idx.reshape(NC_, NB * Q * HCALLS, IDXH, 16)
    A = fi.transpose(0, 3, 1, 2).reshape(NC_, 16, NB * IDXB)
    idx_dram = np.tile(A, (1, 8, 1))  # [8, 128, NB*IDXB]
    dl_dram = (
        flat_dl.reshape(NC_, NB * BT, P).transpose(0, 2, 1).astype(BF16)
    )  # [8, 128, NB*BT]

    dis_sb = np.zeros((NC_, P, G), np.float32)
    dis_sb[node_core, node_slot, node_gabs] = dis
    binv = np.zeros((NC_, G * P), np.float32)
    binv[node_core, node_gabs * P + node_slot] = 1.0 / dis

    return dict(
        xs=xs, xs_own=xs_own, idx=np.ascontiguousarray(idx_dram),
        dl=np.ascontiguousarray(dl_dram), dis_sb=dis_sb, binv=binv, gid=gid
    )


# --------------------------------------------------------------- device side

def build_program():
    f32 = mybir.dt.float32
    bf16 = mybir.dt.bfloat16
    i16 = mybir.dt.int16
    AO = mybir.AluOpType

    nc = bacc.Bacc(
        "TRN2", target_bir_lowering=False, debug=False, num_devices=NC_,
        num_swdge_queues=4, dynamic_dma_scratch_size=32768,
    )
    xs_d = nc.dram_tensor("xs", [TROWS, IN_DIM], bf16, kind="ExternalInput")
    xso_d = nc.dram_tensor("xso", [NODES_PC, IN_DIM], bf16, kind="ExternalInput")
    idx_d = nc.dram_tensor("idx", [P, NB * IDXB], i16, kind="ExternalInput")
    dl_d = nc.dram_tensor("dl", [P, NB * BT], bf16, kind="ExternalInput")
    dis_d = nc.dram_tensor("dis", [P, G], f32, kind="ExternalInput")
    dis2_d = nc.dram_tensor("dis2", [P, G], f32, kind="ExternalInput")
    binv_d = nc.dram_tensor("binv", [1, G * P], bf16, kind="ExternalInput")
    w1_d = nc.dram_tensor("w1", [IN_DIM, HID], bf16, kind="ExternalInput")
    w2_d = nc.dram_tensor("w2", [HID, OUT_DIM], bf16, kind="ExternalInput")
    b1_d = nc.dram_tensor("b1w", [1, HID], bf16, kind="ExternalInput")
    b2_d = nc.dram_tensor("b2w", [1, OUT_DIM], bf16, kind="ExternalInput")
    iota_d = nc.dram_tensor("iota", [P, P], f32, kind="ExternalInput")
    iden_d = nc.dram_tensor("iden", [P, P], bf16, kind="ExternalInput")
    out_d = nc.dram_tensor("out", [NODES_PC, OUT_DIM], f32, kind="ExternalOutput")

    with tile.TileContext(nc) as tc:
        with tc.tile_pool(name="const", bufs=1) as cpool, \
             tc.tile_pool(name="io", bufs=4) as iopool, \
             tc.tile_pool(name="own", bufs=2) as opool, \
             tc.tile_pool(name="msgp", bufs=3) as mpool, \
             tc.tile_pool(name="sp", bufs=2) as spool, \
             tc.tile_pool(name="epi", bufs=3) as epool, \
             tc.tile_pool(name="psag", bufs=3, space="PSUM") as psag, \
             tc.tile_pool(name="psep", bufs=2, space="PSUM") as psep, \
             tc.tile_pool(name="psio", bufs=1, space="PSUM") as psio, \
             tc.tile_pool(name="dram", bufs=1, space="DRAM") as dpool:

            w1s = cpool.tile([IN_DIM, HID], bf16)
            nc.sync.dma_start(out=w1s[:], in_=w1_d[:])
            w2s = cpool.tile([HID, OUT_DIM], bf16)
            nc.sync.dma_start(out=w2s[:], in_=w2_d[:])
            b1s = cpool.tile([1, HID], bf16)
            nc.sync.dma_start(out=b1s[:], in_=b1_d[:])
            b2s = cpool.tile([1, OUT_DIM], bf16)
            nc.sync.dma_start(out=b2s[:], in_=b2_d[:])
            dis_s = cpool.tile([P, G], f32)
            nc.sync.dma_start(out=dis_s[:], in_=dis_d[:])
            dis2_s = cpool.tile([P, G], f32)
            nc.sync.dma_start(out=dis2_s[:], in_=dis2_d[:])
            iota_s = cpool.tile([P, P], f32)
            nc.sync.dma_start(out=iota_s[:], in_=iota_d[:])
            iden_s = cpool.tile([P, P], bf16)
            nc.sync.dma_start(out=iden_s[:], in_=iden_d[:])
            iota_ps = psio.tile([P, P], f32)
            nc.scalar.copy(out=iota_ps[:], in_=iota_s[:])

            gshard = dpool.tile([NODES_PC, HID], bf16)
            gq = [
                dpool.tile([QROWS, HID], bf16, addr_space="Shared",
                           name=f"gq{k}")
                for k in range(Q)
            ]

            def layer(tbl_of_chunk, own_tbl, wsb, bsb, dout, sink,
                      post_batch=None):
                for b in range(NB):
                    idx_t = iopool.tile([P, IDXB], i16, tag="idx")
                    nc.sync.dma_start(
                        out=idx_t[:], in_=idx_d[:, b * IDXB:(b + 1) * IDXB]
                    )
                    dl_t = iopool.tile([P, BT], bf16, tag="dl")
                    nc.sync.dma_start(out=dl_t[:], in_=dl_d[:, b * BT:(b + 1) * BT])
                    binv_t = iopool.tile([1, W * P], bf16, tag="binv")
                    nc.sync.dma_start(
                        out=binv_t[:], in_=binv_d[:, b * W * P:(b + 1) * W * P]
                    )
                    ownT = opool.tile([P, W * P], bf16, tag="ownT")
                    nc.sync.dma_start_transpose(
                        out=ownT[:], in_=own_tbl[b * W * P:(b + 1) * W * P, :]
                    )
                    msg = mpool.tile([P, BT, P], bf16, tag="msg")
                    for h in range(HCALLS):
                        for c in range(Q):
                            t0 = c * SEC_T + h * HT
                            s0 = (c * HCALLS + h) * IDXH
                            nc.gpsimd.dma_gather(
                                out_ap=msg[:, t0:t0 + HT, :],
                                in_ap=tbl_of_chunk(c),
                                idxs_ap=idx_t[:, s0:s0 + IDXH],
                                num_idxs=HT * P,
                                num_idxs_reg=HT * P,
                                elem_size=IN_DIM,
                                elem_step=IN_DIM,
                                single_packet=False,
                                queue_num=c,
                            )
                    S3 = spool.tile([P, BT, P], bf16, tag="S3")
                    nc.vector.tensor_tensor(
                        out=S3[:],
                        in0=dl_t[:].unsqueeze(2).to_broadcast([P, BT, P]),
                        in1=iota_ps[:].unsqueeze(1).to_broadcast([P, BT, P]),
                        op=AO.is_equal,
                    )
                    for g in range(W):
                        gabs = b * W + g
                        ps = psag.tile([P, P], mybir.dt.float32, tag="agg")
                        for c in range(Q):
                            for k in range(TPC):
                                t = c * SEC_T + g * TPC + k
                                nc.tensor.matmul(
                                    out=ps[:],
                                    lhsT=msg[:, t, :],
                                    rhs=S3[:, t, :],
                                    start=(c == 0 and k == 0),
                                    stop=False,
                                )
                        # self-loop: ps += ownT[:, g] (identity-matmul add)
                        nc.tensor.matmul(
                            out=ps[:],
                            lhsT=iden_s[:],
                            rhs=ownT[:, g * P:(g + 1) * P],
                            start=False, stop=True,
                        )
                        aggT = epool.tile([P, P], bf16, tag="aggT")
                        nc.scalar.copy(out=aggT[:], in_=ps[:])
                        po = psep.tile([P, dout], mybir.dt.float32, tag="po")
                        # bias pre-load: po = (b / dis)[dst, f] via outer product
                        nc.tensor.matmul(
                            out=po[:],
                            lhsT=binv_t[:, g * P:(g + 1) * P],
                            rhs=bsb[:],
                            start=True, stop=False,
                        )
                        nc.tensor.matmul(
                            out=po[:], lhsT=aggT[:], rhs=wsb[:], start=False, stop=True
                        )
                        sink(gabs, po)
                    if post_batch is not None:
                        post_batch(b)

            def sink1(gabs, po):
                # dis*relu(dis*agg + b1) == relu(dis2*(agg + b1/dis))
                gt = epool.tile([P, HID], mybir.dt.bfloat16, tag="gt")
                nc.scalar.activation(
                    out=gt[:], in_=po[:],
                    func=mybir.ActivationFunctionType.Relu,
                    scale=dis2_s[:, gabs:gabs + 1],
                )
                nc.sync.dma_start(
                    out=gshard[gabs * P:(gabs + 1) * P, :], in_=gt[:]
                )

            def sink2(gabs, po):
                # dis*agg + b2 == Copy(dis*(agg + b2/dis))
                o = epool.tile([P, OUT_DIM], mybir.dt.float32, tag="o")
                nc.scalar.activation(
                    out=o[:], in_=po[:],
                    func=mybir.ActivationFunctionType.Copy,
                    scale=dis_s[:, gabs:gabs + 1],
                )
                nc.sync.dma_start(
                    out=out_d[gabs * P:(gabs + 1) * P, :], in_=o[:]
                )

            def ag_piece(b):
                # fire AllGather piece k (== quarter k) once its groups are
                # sunk; output lands directly in the quarter sub-table
                if (b + 1) not in PIECE_B:
                    return
                k = PIECE_B.index(b + 1)
                r0, r1 = k * GQ * P, (k + 1) * GQ * P
                nc.gpsimd.collective_compute(
                    "AllGather",
                    mybir.AluOpType.bypass,
                    replica_groups=[list(range(NC_))],
                    ins=[gshard[r0:r1, :].opt()],
                    outs=[gq[k].opt()],
                )

            layer(lambda c: xs_d[c * QROWS:(c + 1) * QROWS, :], xso_d[:],
                  w1s, b1s, HID, sink1, post_batch=ag_piece)
            layer(lambda c: gq[c][:], gshard[:],
                  w2s, b2s, OUT_DIM, sink2)

    nc.compile()
    return nc


# ------------------------------------------------------------------- runner

def run(inputs, trace=False):
    global _compiled
    x = np.asarray(inputs["x"], np.float32)
    edge_index = np.asarray(inputs["edge_index"])
    W1 = np.asarray(inputs["W1"], np.float32)
    b1 = np.asarray(inputs["b1"], np.float32)
    W2 = np.asarray(inputs["W2"], np.float32)
    b2 = np.asarray(inputs["b2"], np.float32)

    pp = preprocess(x, edge_index)

    if _compiled is None:
        _compiled = build_program()
    nc = _compiled

    iota = np.ascontiguousarray(
        np.broadcast_to(np.arange(P, dtype=np.float32), (P, P))
    )
    iden = np.eye(P, dtype=BF16)
    w1b = W1.astype(BF16)
    w2b = W2.astype(BF16)
    b1w = b1.reshape(1, HID).astype(BF16)
    b2w = b2.reshape(1, OUT_DIM).astype(BF16)

    in_maps = []
    for c in range(NC_):
        in_maps.append({
            "xs": pp["xs"],
            "xso": pp["xs_own"][c],
            "idx": pp["idx"][c],
            "dl": pp["dl"][c],
            "dis": pp["dis_sb"][c],
            "dis2": pp["dis_sb"][c] ** 2,
            "binv": pp["binv"][c].reshape(1, G * P).astype(BF16),
            "w1": w1b,
            "w2": w2b,
            "b1w": b1w,
            "b2w": b2w,
            "iota": iota,
            "iden": iden,
        })

    res = run_bass_kernel_spmd(
        nc, in_maps, core_ids=list(range(NC_)), trace=trace
    )
    allf = np.concatenate([res.results[c]["out"] for c in range(NC_)], axis=0)
    out = allf[pp["gid"]].astype(np.float32)
    return out, res


def kernel(**inputs):
    out, _ = run(inputs, trace=False)
    return out


# revision 19
# speedup vs baseline: 1.0804x; 1.0275x over previous
"""2-layer GCN encoder on 8 Trainium2 NeuronCores (Bass/Tile).

Math: with dis = deg^{-1/2} (self-loops included), the GCN layer
    out = relu(D^{-1/2} A D^{-1/2} (X W) + b)
separates as
    out[v] = relu(dis[v] * (sum_{e: dst=v} dis[src]*X[src]) @ W + b)
so the per-edge norm disappears and both weight matmuls commute out of the
edge aggregation.  Aggregation is done as binary-selection matmuls on the
TensorEngine over dma_gather'ed rows of the dis-prescaled feature table.

Sharding: nodes are bin-packed by in-degree into 8 cores x 104 groups x 128
slots.  Groups are partitioned into Q=4 quarters of 26 groups per core; a
node's gather-chunk is the quarter of its group, and the (dst group, src
quarter) cell loads are balanced to <= 512 edges by bin- and node-level
swaps so every cell fits TPC=4 tiles of 128.  The table is quarter-major:
each quarter is one contiguous sub-table of 26624 rows (int16-indexable),
and the layer-2 table is four Shared DRAM tensors, each written in place by
exactly one pipelined AllGather piece -- no post-collective scatter.

Self-loops are excluded from the edge schedule: each group's own rows are
bulk-loaded transposed (one xbar-transpose DMA per batch) and accumulated
into the aggregation PSUM via an identity-matrix matmul, saving ~6% of the
per-element gather descriptors.

Device-side notes:
  - the one-hot selection matrix S3 is built on DVE with the iota operand in
    PSUM so the op uses only dedicated SBUF ports and never locks GpSimd
    (SWDGE descriptor generation) out of the shared port pair;
  - bias is pre-loaded into PSUM as outer(1/dis, b) via a K=1 matmul, and the
    dst-side dis scaling + relu ride the Scalar engine's activation
    (dis*relu(x) == relu(dis*x));
  - each chunk's gather is split into two half-calls issued wave-wise
    (h-major) so a descriptor-ring await on one SWDGE queue never blocks the
    other queues' in-order decode on the Pool engine.
"""

import numpy as np
import ml_dtypes

import concourse.bacc as bacc
import concourse.tile as tile
import concourse.mybir as mybir
import concourse.bass as bass
from concourse.bass_utils import run_bass_kernel_spmd

# problem shapes (hardcoded per contract)
N = 100000
E = 1600000
IN_DIM, HID, OUT_DIM = 128, 128, 64

# schedule constants
P = 128           # partitions / tile edge count
NC_ = 8           # cores
G = 104           # groups per core
W = 8             # groups per batch
NB = 13           # batches per layer (W*NB == G)
Q = 4             # quarters (gather chunks == AllGather pieces)
GQ = G // Q       # groups per quarter per core  = 26
TPC = 4           # tiles per (group, chunk) cell
CELL_CAP = TPC * P  # max edges per (dst group, src quarter) cell = 512
SEC_T = W * TPC   # tiles per chunk section per batch = 32
BT = Q * SEC_T    # tiles per batch                   = 128
NODES_PC = G * P  # padded nodes per core             = 13312
QROWS = NC_ * GQ * P    # rows per quarter sub-table  = 26624
TROWS = NC_ * NODES_PC  # shared table rows           = 106496
HCALLS = 2              # gather half-calls per chunk
HT = SEC_T // HCALLS    # tiles per half-call = 16
IDXH = HT * P // 16     # wrapped idx cols per half-call = 128
IDXB = Q * HCALLS * IDXH  # idx cols per batch = 1024
# AllGather piece k (== quarter k) fires once batches covering its 26 groups
# are sunk: after batch indices 3, 6, 9, 12
PIECE_B = [4, 7, 10, 13]

BF16 = ml_dtypes.bfloat16

_compiled = None  # cache across calls


# ----------------------------------------------------------------- host side

def _pack_and_balance(src, dst, deg_in):
    """Assign nodes to 8*G bins (<=128 each) and bins to (core, quarter) so
    that every (dst bin, src quarter) cell has <= CELL_CAP edges.

    Returns bin_of [N], core_of_bin, quarter_of_bin [NBINS].
    """
    NBINS = NC_ * G
    rng = np.random.default_rng(0)

    # snake-deal nodes by in-degree: balances per-bin degree loads
    order = np.argsort(-deg_in, kind="stable")
    seq = np.arange(N) % (2 * NBINS)
    binpos = np.where(seq < NBINS, seq, 2 * NBINS - 1 - seq)
    bin_of = np.empty(N, np.int64)
    bin_of[order] = binpos
    counts = np.bincount(bin_of, minlength=NBINS)
    assert counts.max() <= P

    core_of_bin = np.arange(NBINS) % NC_
    within = np.arange(NBINS) // NC_
    qq = within % (2 * Q)
    qob = np.where(qq < Q, qq, 2 * Q - 1 - qq)

    B = np.zeros((NBINS, NBINS), np.int32)
    np.add.at(B, (bin_of[src], bin_of[dst]), 1)

    def cellmat():
        c = np.zeros((Q, NBINS), np.int64)
        for q in range(Q):
            c[q] = B[qob == q].sum(axis=0)
        return c

    c = cellmat()

    # --- bin phase: swap quarters of two same-core bins ---
    for _ in range(5000):
        exc = np.maximum(c - CELL_CAP, 0)
        if exc.sum() == 0:
            break
        q, dd = np.unravel_index(np.argmax(exc), exc.shape)
        cand = np.flatnonzero(qob == q)
        cand = cand[np.argsort(-B[cand, dd])][:3]
        best = (0, -1, -1)
        for b in cand:
            mates = np.flatnonzero((core_of_bin == core_of_bin[b]) & (qob != q))
            rowb = B[b]
            for b2 in mates:
                q2 = qob[b2]
                diff = rowb - B[b2]
                delta = (
                    np.maximum(c[q] - diff - CELL_CAP, 0).sum()
                    + np.maximum(c[q2] + diff - CELL_CAP, 0).sum()
                    - exc[q].sum() - exc[q2].sum()
                )
                if delta < best[0]:
                    best = (delta, b, b2)
        if best[1] < 0:
            break
        _, b, b2 = best
        q2 = qob[b2]
        diff = B[b] - B[b2]
        c[q] -= diff
        c[q2] += diff
        qob[b], qob[b2] = q2, q

    # --- node phase: swap two nodes between bins of different quarters ---
    os_ = np.argsort(src, kind="stable")
    osrc_sorted = src[os_]
    odst_node = dst[os_]
    optr = np.searchsorted(osrc_sorted, np.arange(N + 1))
    is_ = np.argsort(dst, kind="stable")
    idst_sorted = dst[is_]
    isrc_node = src[is_]
    iptr = np.searchsorted(idst_sorted, np.arange(N + 1))

    nodes_by_bin = [np.flatnonzero(bin_of == b) for b in range(NBINS)]

    def try_swap(u, v2, c, strict):
        bu, b2 = bin_of[u], bin_of[v2]
        q, q2 = qob[bu], qob[b2]
        du = bin_of[odst_node[optr[u]:optr[u + 1]]]
        dv = bin_of[odst_node[optr[v2]:optr[v2 + 1]]]
        iu = bin_of[isrc_node[iptr[u]:iptr[u + 1]]]
        iv = bin_of[isrc_node[iptr[v2]:iptr[v2 + 1]]]
        cn = c.copy()
        np.add.at(cn[q], du, -1)
        np.add.at(cn[q2], du, 1)
        np.add.at(cn[q2], dv, -1)
        np.add.at(cn[q], dv, 1)
        np.add.at(cn, (qob[iu], np.full(len(iu), bu)), -1)
        np.add.at(cn, (qob[iu], np.full(len(iu), b2)), 1)
        np.add.at(cn, (qob[iv], np.full(len(iv), b2)), -1)
        np.add.at(cn, (qob[iv], np.full(len(iv), bu)), 1)
        d = np.maximum(cn - CELL_CAP, 0).sum() - np.maximum(c - CELL_CAP, 0).sum()
        if d < 0 or (not strict and d == 0):
            return cn
        return None

    moves = 0
    for it in range(60000):
        exc = np.maximum(c - CELL_CAP, 0)
        tot = exc.sum()
        if tot == 0:
            break
        q, dd = np.unravel_index(np.argmax(exc), exc.shape)
        # src nodes feeding cell (dd, q), weighted by multiplicity
        cnt = {}
        for v in nodes_by_bin[dd]:
            for e in range(iptr[v], iptr[v + 1]):
                u = isrc_node[e]
                if qob[bin_of[u]] == q:
                    cnt[u] = cnt.get(u, 0) + 1
        us = sorted(cnt, key=cnt.get, reverse=True)[:6]
        done = False
        for strict in (True, False):
            for u in us:
                q2s = np.argsort(c[:, dd])
                for q2 in q2s:
                    if q2 == q:
                        continue
                    tb = np.flatnonzero(qob == q2)
                    for b2 in rng.choice(tb, size=min(6, len(tb)), replace=False):
                        vc = nodes_by_bin[b2]
                        v2 = vc[np.argmin(np.abs(deg_in[vc] - deg_in[u]))]
                        cn = try_swap(u, v2, c, strict)
                        if cn is not None:
                            bu, b2_ = bin_of[u], bin_of[v2]
                            c = cn
                            bin_of[u], bin_of[v2] = b2_, bu
                            nodes_by_bin[bu] = nodes_by_bin[bu][nodes_by_bin[bu] != u]
                            nodes_by_bin[bu] = np.append(nodes_by_bin[bu], v2)
                            nodes_by_bin[b2_] = nodes_by_bin[b2_][nodes_by_bin[b2_] != v2]
                            nodes_by_bin[b2_] = np.append(nodes_by_bin[b2_], u)
                            moves += 1
                            done = True
                            break
                    if done:
                        break
                if done:
                    break
            if done:
                break
        if not done:
            break
        if moves % 256 == 0:
            # guard against incremental drift (adjacent-node swaps)
            Bn = np.zeros((NBINS, NBINS), np.int32)
            np.add.at(Bn, (bin_of[src], bin_of[dst]), 1)
            B = Bn
            c = cellmat()

    # authoritative final check
    B = np.zeros((NBINS, NBINS), np.int32)
    np.add.at(B, (bin_of[src], bin_of[dst]), 1)
    c = cellmat()
    assert np.maximum(c - CELL_CAP, 0).sum() == 0, (
        f"cell balance failed: max={c.max()}"
    )
    return bin_of, core_of_bin, qob


def preprocess(x, edge_index):
    src = np.asarray(edge_index[0], dtype=np.int64)
    dst = np.asarray(edge_index[1], dtype=np.int64)
    deg = np.bincount(dst, minlength=N).astype(np.float64) + 1.0  # + self-loop
    dis = (1.0 / np.sqrt(deg)).astype(np.float32)

    deg_in = deg.astype(np.int64)
    bin_of, core_of_bin, qob = _pack_and_balance(src, dst, deg_in)
    NBINS = NC_ * G

    # order bins within each (core, quarter) -> gabs
    gabs_of_bin = np.empty(NBINS, np.int64)
    for core in range(NC_):
        for q in range(Q):
            sel = np.flatnonzero((core_of_bin == core) & (qob == q))
            assert len(sel) == GQ
            gabs_of_bin[sel] = q * GQ + np.arange(GQ)

    node_core = core_of_bin[bin_of]
    node_gabs = gabs_of_bin[bin_of]
    # slot within bin: arbitrary stable order
    bkey = bin_of
    order = np.argsort(bkey, kind="stable")
    sorted_b = bkey[order]
    starts = np.searchsorted(sorted_b, np.arange(NBINS + 1))
    rnk = np.arange(N) - np.repeat(starts[:-1], np.diff(starts))
    node_slot = np.empty(N, np.int64)
    node_slot[order] = rnk
    assert node_slot.max() < P

    # output gather order: core-major (matches per-core out tensors)
    gid = node_core * NODES_PC + node_gabs * P + node_slot
    # shared table order: quarter-major so each AllGather piece (quarter) is
    # one contiguous sub-table written in place by its collective
    nq = node_gabs // GQ
    tix = (
        nq * QROWS
        + node_core * (GQ * P)
        + (node_gabs - nq * GQ) * P
        + node_slot
    )

    # layer-1 table in tix order + per-core own-block table in gabs order
    xpre = (np.asarray(x, np.float32) * dis[:, None]).astype(BF16)
    xs = np.zeros((TROWS, IN_DIM), BF16)
    xs[tix] = xpre
    xs_own = np.zeros((NC_, NODES_PC, IN_DIM), BF16)
    xs_own[node_core, node_gabs * P + node_slot] = xpre
    # feature-major so per-batch own loads are 2KB-contiguous per partition
    xs_own_t = np.ascontiguousarray(xs_own.transpose(0, 2, 1))

    # shared edge schedule (same for both layers); self-loops excluded
    ecore = node_core[dst]
    egabs = node_gabs[dst]
    eslot = node_slot[dst]
    src_tix = tix[src]
    chunk = src_tix // QROWS
    eidx = src_tix % QROWS
    cell = (ecore * G + egabs) * Q + chunk
    order = np.lexsort((eidx, cell))
    cell_s = cell[order]
    counts = np.bincount(cell, minlength=NBINS * Q)
    assert counts.max() <= CELL_CAP
    starts = np.concatenate([[0], np.cumsum(counts)[:-1]])
    rank = np.arange(len(cell_s)) - np.repeat(starts, counts)
    ch = cell_s % Q
    gg = (cell_s // Q) % G
    cr = cell_s // (Q * G)
    batch = gg // W
    gslot = gg % W
    tile_k = rank // P
    pos = rank % P
    T = batch * BT + ch * SEC_T + gslot * TPC + tile_k
    goff = cr * (NB * BT * P) + T * P + pos
    flat_idx = np.zeros(NC_ * NB * BT * P, np.int16)
    flat_dl = np.full(NC_ * NB * BT * P, P, np.int16)
    flat_idx[goff] = eidx[order].astype(np.int16)
    flat_dl[goff] = eslot[order].astype(np.int16)

    # wrapped idx layout: wrapped[p, s] = flat[s*16 + p%16], replicated x8.
    fi = flat_idx.reshape(NC_, NB * Q * HCALLS, IDXH, 16)
    A = fi.transpose(0, 3, 1, 2).reshape(NC_, 16, NB * IDXB)
    idx_dram = np.tile(A, (1, 8, 1))  # [8, 128, NB*IDXB]
    dl_dram = (
        flat_dl.reshape(NC_, NB * BT, P).transpose(0, 2, 1).astype(BF16)
    )  # [8, 128, NB*BT]

    dis_sb = np.zeros((NC_, P, G), np.float32)
    dis_sb[node_core, node_slot, node_gabs] = dis
    binv = np.zeros((NC_, G * P), np.float32)
    binv[node_core, node_gabs * P + node_slot] = 1.0 / dis

    return dict(
        xs=xs, xs_own_t=xs_own_t, idx=np.ascontiguousarray(idx_dram),
        dl=np.ascontiguousarray(dl_dram), dis_sb=dis_sb, binv=binv, gid=gid
    )


# --------------------------------------------------------------- device side

def build_program():
    f32 = mybir.dt.float32
    bf16 = mybir.dt.bfloat16
    i16 = mybir.dt.int16
    AO = mybir.AluOpType

    nc = bacc.Bacc(
        "TRN2", target_bir_lowering=False, debug=False, num_devices=NC_,
        num_swdge_queues=4, dynamic_dma_scratch_size=49152,
    )
    xs_d = nc.dram_tensor("xs", [TROWS, IN_DIM], bf16, kind="ExternalInput")
    xso_d = nc.dram_tensor("xso", [IN_DIM, NODES_PC], bf16, kind="ExternalInput")
    idx_d = nc.dram_tensor("idx", [P, NB * IDXB], i16, kind="ExternalInput")
    dl_d = nc.dram_tensor("dl", [P, NB * BT], bf16, kind="ExternalInput")
    dis_d = nc.dram_tensor("dis", [P, G], f32, kind="ExternalInput")
    dis2_d = nc.dram_tensor("dis2", [P, G], f32, kind="ExternalInput")
    binv_d = nc.dram_tensor("binv", [1, G * P], bf16, kind="ExternalInput")
    w1_d = nc.dram_tensor("w1", [IN_DIM, HID], bf16, kind="ExternalInput")
    w2_d = nc.dram_tensor("w2", [HID, OUT_DIM], bf16, kind="ExternalInput")
    b1_d = nc.dram_tensor("b1w", [1, HID], bf16, kind="ExternalInput")
    b2_d = nc.dram_tensor("b2w", [1, OUT_DIM], bf16, kind="ExternalInput")
    iota_d = nc.dram_tensor("iota", [P, P], f32, kind="ExternalInput")
    iden_d = nc.dram_tensor("iden", [P, P], bf16, kind="ExternalInput")
    out_d = nc.dram_tensor("out", [NODES_PC, OUT_DIM], f32, kind="ExternalOutput")

    with tile.TileContext(nc) as tc:
        with tc.tile_pool(name="const", bufs=1) as cpool, \
             tc.tile_pool(name="io", bufs=4) as iopool, \
             tc.tile_pool(name="own", bufs=2) as opool, \
             tc.tile_pool(name="msgp", bufs=2) as mpool, \
             tc.tile_pool(name="sp", bufs=2) as spool, \
             tc.tile_pool(name="epi", bufs=3) as epool, \
             tc.tile_pool(name="psag", bufs=3, space="PSUM") as psag, \
             tc.tile_pool(name="psep", bufs=2, space="PSUM") as psep, \
             tc.tile_pool(name="psio", bufs=1, space="PSUM") as psio, \
             tc.tile_pool(name="dram", bufs=1, space="DRAM") as dpool:

            w1s = cpool.tile([IN_DIM, HID], bf16)
            nc.sync.dma_start(out=w1s[:], in_=w1_d[:])
            w2s = cpool.tile([HID, OUT_DIM], bf16)
            nc.sync.dma_start(out=w2s[:], in_=w2_d[:])
            b1s = cpool.tile([1, HID], bf16)
            nc.sync.dma_start(out=b1s[:], in_=b1_d[:])
            b2s = cpool.tile([1, OUT_DIM], bf16)
            nc.sync.dma_start(out=b2s[:], in_=b2_d[:])
            dis_s = cpool.tile([P, G], f32)
            nc.sync.dma_start(out=dis_s[:], in_=dis_d[:])
            dis2_s = cpool.tile([P, G], f32)
            nc.sync.dma_start(out=dis2_s[:], in_=dis2_d[:])
            iota_s = cpool.tile([P, P], f32)
            nc.sync.dma_start(out=iota_s[:], in_=iota_d[:])
            iden_s = cpool.tile([P, P], bf16)
            nc.sync.dma_start(out=iden_s[:], in_=iden_d[:])
            iota_ps = psio.tile([P, P], f32)
            nc.scalar.copy(out=iota_ps[:], in_=iota_s[:])

            gshard = dpool.tile([NODES_PC, HID], bf16)
            gq = [
                dpool.tile([QROWS, HID], bf16, name=f"gq{k}")
                for k in range(Q)
            ]

            def layer(tbl_of_chunk, own_load, wsb, bsb, dout, sink,
                      post_batch=None):
                for b in range(NB):
                    idx_t = iopool.tile([P, IDXB], i16, tag="idx")
                    nc.sync.dma_start(
                        out=idx_t[:], in_=idx_d[:, b * IDXB:(b + 1) * IDXB]
                    )
                    dl_t = iopool.tile([P, BT], bf16, tag="dl")
                    nc.sync.dma_start(out=dl_t[:], in_=dl_d[:, b * BT:(b + 1) * BT])
                    binv_t = iopool.tile([1, W * P], bf16, tag="binv")
                    nc.sync.dma_start(
                        out=binv_t[:], in_=binv_d[:, b * W * P:(b + 1) * W * P]
                    )
                    ownT = opool.tile([P, W * P], bf16, tag="ownT")
                    own_load(b, ownT)
                    msg = mpool.tile([P, BT, P], bf16, tag="msg")
                    for h in range(HCALLS):
                        for c in range(Q):
                            t0 = c * SEC_T + h * HT
                            s0 = (c * HCALLS + h) * IDXH
                            nc.gpsimd.dma_gather(
                                out_ap=msg[:, t0:t0 + HT, :],
                                in_ap=tbl_of_chunk(c),
                                idxs_ap=idx_t[:, s0:s0 + IDXH],
                                num_idxs=HT * P,
                                num_idxs_reg=HT * P,
                                elem_size=IN_DIM,
                                elem_step=IN_DIM,
                                single_packet=False,
                                queue_num=c,
                            )
                    S3 = spool.tile([P, BT, P], bf16, tag="S3")
                    nc.vector.tensor_tensor(
                        out=S3[:],
                        in0=dl_t[:].unsqueeze(2).to_broadcast([P, BT, P]),
                        in1=iota_ps[:].unsqueeze(1).to_broadcast([P, BT, P]),
                        op=AO.is_equal,
                    )
                    for g in range(W):
                        gabs = b * W + g
                        ps = psag.tile([P, P], mybir.dt.float32, tag="agg")
                        for c in range(Q):
                            for k in range(TPC):
                                t = c * SEC_T + g * TPC + k
                                nc.tensor.matmul(
                                    out=ps[:],
                                    lhsT=msg[:, t, :],
                                    rhs=S3[:, t, :],
                                    start=(c == 0 and k == 0),
                                    stop=False,
                                )
                        # self-loop: ps += ownT[:, g] (identity-matmul add)
                        nc.tensor.matmul(
                            out=ps[:],
                            lhsT=iden_s[:],
                            rhs=ownT[:, g * P:(g + 1) * P],
                            start=False, stop=True,
                        )
                        aggT = epool.tile([P, P], bf16, tag="aggT")
                        nc.scalar.copy(out=aggT[:], in_=ps[:])
                        po = psep.tile([P, dout], mybir.dt.float32, tag="po")
                        # bias pre-load: po = (b / dis)[dst, f] via outer product
                        nc.tensor.matmul(
                            out=po[:],
                            lhsT=binv_t[:, g * P:(g + 1) * P],
                            rhs=bsb[:],
                            start=True, stop=False,
                        )
                        nc.tensor.matmul(
                            out=po[:], lhsT=aggT[:], rhs=wsb[:], start=False, stop=True
                        )
                        sink(gabs, po)
                    if post_batch is not None:
                        post_batch(b)

            def sink1(gabs, po):
                # dis*relu(dis*agg + b1) == relu(dis2*(agg + b1/dis))
                gt = epool.tile([P, HID], mybir.dt.bfloat16, tag="gt")
                nc.scalar.activation(
                    out=gt[:], in_=po[:],
                    func=mybir.ActivationFunctionType.Relu,
                    scale=dis2_s[:, gabs:gabs + 1],
                )
                nc.sync.dma_start(
                    out=gshard[gabs * P:(gabs + 1) * P, :], in_=gt[:]
                )

            def sink2(gabs, po):
                # dis*agg + b2 == Copy(dis*(agg + b2/dis))
                o = epool.tile([P, OUT_DIM], mybir.dt.float32, tag="o")
                nc.scalar.activation(
                    out=o[:], in_=po[:],
                    func=mybir.ActivationFunctionType.Copy,
                    scale=dis_s[:, gabs:gabs + 1],
                )
                nc.sync.dma_start(
                    out=out_d[gabs * P:(gabs + 1) * P, :], in_=o[:]
                )

            def ag_piece(b):
                # fire AllGather piece k (== quarter k) once its groups are
                # sunk; output lands directly in the quarter sub-table
                if (b + 1) not in PIECE_B:
                    return
                k = PIECE_B.index(b + 1)
                r0, r1 = k * GQ * P, (k + 1) * GQ * P
                nc.gpsimd.collective_compute(
                    "AllGather",
                    mybir.AluOpType.bypass,
                    replica_groups=[list(range(NC_))],
                    ins=[gshard[r0:r1, :].opt()],
                    outs=[gq[k].opt()],
                )

            def own1(b, ownT):
                nc.sync.dma_start(
                    out=ownT[:], in_=xso_d[:, b * W * P:(b + 1) * W * P]
                )

            def own2(b, ownT):
                nc.sync.dma_start_transpose(
                    out=ownT[:], in_=gshard[b * W * P:(b + 1) * W * P, :]
                )

            layer(lambda c: xs_d[c * QROWS:(c + 1) * QROWS, :], own1,
                  w1s, b1s, HID, sink1, post_batch=ag_piece)
            layer(lambda c: gq[c][:], own2,
                  w2s, b2s, OUT_DIM, sink2)

    nc.compile()
    return nc


# ------------------------------------------------------------------- runner

def run(inputs, trace=False):
    global _compiled
    x = np.asarray(inputs["x"], np.float32)
    edge_index = np.asarray(inputs["edge_index"])
    W1 = np.asarray(inputs["W1"], np.float32)
    b1 = np.asarray(inputs["b1"], np.float32)
    W2 = np.asarray(inputs["W2"], np.float32)
    b2 = np.asarray(inputs["b2"], np.float32)

    pp = preprocess(x, edge_index)

    if _compiled is None:
        _compiled = build_program()
    nc = _compiled

    iota = np.ascontiguousarray(
        np.broadcast_to(np.arange(P, dtype=np.float32), (P, P))
    )
    iden = np.eye(P, dtype=BF16)
    w1b = W1.astype(BF16)
    w2b = W2.astype(BF16)
    b1w = b1.reshape(1, HID).astype(BF16)
    b2w = b2.reshape(1, OUT_DIM).astype(BF16)

    in_maps = []
    for c in range(NC_):
        in_maps.append({
            "xs": pp["xs"],
            "xso": pp["xs_own_t"][c],
            "idx": pp["idx"][c],
            "dl": pp["dl"][c],
            "dis": pp["dis_sb"][c],
            "dis2": pp["dis_sb"][c] ** 2,
            "binv": pp["binv"][c].reshape(1, G * P).astype(BF16),
            "w1": w1b,
            "w2": w2b,
            "b1w": b1w,
            "b2w": b2w,
            "iota": iota,
            "iden": iden,
        })

    res = run_bass_kernel_spmd(
        nc, in_maps, core_ids=list(range(NC_)), trace=trace
    )
    allf = np.concatenate([res.results[c]["out"] for c in range(NC_)], axis=0)
    out = allf[pp["gid"]].astype(np.float32)
    return out, res


def kernel(**inputs):
    out, _ = run(inputs, trace=False)
    return out
